# revision 4
# baseline (speedup 1.0000x reference)
"""GraphAttentionLayer (GAT) Bass kernel for Trainium2, 8 NeuronCores — v2.

Problem: B=8, N=2048, Fin=256, Fout=64
    Wh  = h @ W                                   [B, N, 64]
    e   = Wh@a1 + (Wh@a2)^T  (additive scores)    [B, N, N]
    att = where(adj>0, leaky_relu(e, 0.2), -9e15)
    A   = softmax(att, axis=1)   (column softmax!)
    out = elu(A @ Wh)

Sharding: batch-parallel, one graph per core (no communication).

Design vs the accum-DMA baseline (measured ~6-15 ms/iter on HW; this
kernel measures ~78 us/iter):
  * adjacency mask is preloaded host-side as fp16 {0, -57344} and applied
    additively AFTER the leaky-relu (leaky(e) - 57344 <= -57000 so exp()
    still yields exactly 0 for masked entries).  The mask lands via plain
    full-bandwidth HWDGE loads (alternating between the SP and ACT DMA
    queues) + one DVE tensor_tensor add — the slow GPSIMD accumulate-DMA
    is gone entirely.
  * leaky-relu is band-split: columns [0:C_ACT) run as a single fused ACT
    Prelu (bias = per-partition Wh2), the rest on DVE as two 4x-mode
    tensor_scalar ops + one 2x-mode tensor_tensor max (fp16 operands).
  * the m-tile loop is software-pipelined (front stages of tile mt overlap
    back stages of tile mt-1) so ACT never stalls on the per-tile
    ACT->DVE->ACT round trip.
  * exp output P and the folded Wh operand are bf16 (wide exponent: no
    overflow for exp(+20), no subnormal flush at the softmax tail), making
    mm2 a full-rate 1-cycle/row PE matmul. mm1 runs in fp16.
  * ELU tail uses elu(x) = max(x, min(exp(x)-1, 0)) — one ACT pass + two
    fp16 DVE passes; output is stored fp16 and widened on the host.

Per-core dataflow (transposed layout, m = attended-over node on partitions,
n = output node along free axis; m-tiles of 128):
    mm1 (PE):   whps[m, 0:64] = hT16.T @ W16 ; col 64 = Wh@a2   (PSUM f32)
    DMA (SP):   mk[m, n] <- maskT fp16 {0, -57344}
    leaky:      l[:, 0:C]  = ACT Prelu(wh1b16 + wh2)
                l[:, C:]   = DVE max(t, 0.2t), t = wh1b16 + wh2
    mask (DVE): att = l + mk                     (tensor_tensor, 2x mode)
    ACT:        P = Exp(att) -> bf16, accum_out -> den[m] (f32)
    DVE:        rc = 1/den ; whp = Wh[m, 0:64] * rc -> bf16
    mm2 (PE):   out_T[o, n] += whp.T @ P   (accumulate over 16 m-tiles)
    elu tail:   out = max(x, min(exp(x)-1, 0))   (f32)
Host: transposes h/adj per batch, encodes adj as fp16 mask, transposes the
output back.

Exactness: softmax without max-subtraction is algebraically identical (exp
stays inside f32/bf16 range), masked entries produce exp(l - 57344) == 0
exactly, and the 1/den fold is applied to the contraction operand Wh.
"""

import sys

import numpy as np

if "/opt/trn_rl_repo" not in sys.path:
    sys.path.append("/opt/trn_rl_repo")

import concourse.bass as bass
import concourse.bacc as bacc
import concourse.mybir as mybir
import concourse.tile as tile
from concourse import bass_utils

B = 8
N = 2048
FIN = 256
FOUT = 64
NT = N // 128          # 16 m-tiles
ALPHA = 0.2
MASK = 57344.0         # exp(x - 57344) == 0 for x in leaky-score range
C_ACT = 1152           # leaky-relu columns fused into ACT Prelu; rest on DVE
POOL_Y = 0             # mask-add columns handled by gpsimd (0 = all on DVE)
DT = mybir.dt.float32
F16 = mybir.dt.float16
BF16 = mybir.dt.bfloat16
AF = mybir.ActivationFunctionType
ALU = mybir.AluOpType

_CACHE = {}


def build_program(reps: int = 1, loop_k: int = 0, c_act: int = C_ACT, pool_y: int = POOL_Y,
                  pool_small: bool = False, diag: str = "", lag: int = 1,
                  dual_q: bool = True):
    """Build and compile the SPMD single-core program (identical on 8 cores).

    reps statically unrolls the main body; loop_k wraps it in a dynamic
    For_i loop instead (constant program size -- used for timing).
    """
    nc = bacc.Bacc(
        "TRN2",
        target_bir_lowering=False,
        debug=False,
        enable_asserts=False,
        num_devices=B,
    )
    hT_d = nc.dram_tensor("hT", [FIN, N], DT, kind="ExternalInput")
    W_d = nc.dram_tensor("W", [FIN, FOUT], DT, kind="ExternalInput")
    arow_d = nc.dram_tensor("arow", [1, 2 * FOUT], DT, kind="ExternalInput")
    maskT_d = nc.dram_tensor("maskT", [N, N], F16, kind="ExternalInput")
    out_d = nc.dram_tensor("out", [FOUT, N], F16, kind="ExternalOutput")

    with tile.TileContext(nc) as tc:
        with (
            tc.tile_pool(name="const", bufs=1) as const,
            tc.tile_pool(name="psmall", bufs=3, space=bass.MemorySpace.PSUM) as psmall,
            tc.tile_pool(name="pbig", bufs=1, space=bass.MemorySpace.PSUM) as pbig,
            tc.tile_pool(name="mpool", bufs=3) as mpool,
            tc.tile_pool(name="tpool", bufs=2) as tpool,
            tc.tile_pool(name="lpool", bufs=2) as lpool,
            tc.tile_pool(name="apool", bufs=2) as apool,
            tc.tile_pool(name="ppool", bufs=3) as ppool,
            tc.tile_pool(name="wsm", bufs=4) as wsm,
            tc.tile_pool(name="wout", bufs=1) as wout,
        ):
            # ---- load inputs ----
            hT = [const.tile([128, N], DT, name=f"hT{i}", tag=f"hT{i}") for i in range(2)]
            Wsb = [const.tile([128, FOUT], DT, name=f"W{i}", tag=f"W{i}") for i in range(2)]
            arow = const.tile([1, 2 * FOUT], DT, name="arow", tag="arow")
            for i in range(2):
                nc.sync.dma_start(hT[i][:], hT_d.ap()[i * 128:(i + 1) * 128, :])
                nc.sync.dma_start(Wsb[i][:], W_d.ap()[i * 128:(i + 1) * 128, :])
            nc.sync.dma_start(arow[:], arow_d.ap())

            # fp16 copies for the mm1 operands (1-cycle/row PE path)
            hT16 = [const.tile([128, N], F16, name=f"hT16_{i}", tag=f"hT16_{i}") for i in range(2)]
            for i in range(2):
                nc.vector.tensor_copy(hT16[i][:], hT[i][:])

            # ---- a broadcast + wa vectors ----
            abc = const.tile([128, 2 * FOUT], DT, name="abc", tag="abc")
            nc.gpsimd.partition_broadcast(abc[:], arow[0:1, :])
            wa1 = [const.tile([128, 1], DT, name=f"wa1_{i}", tag=f"wa1_{i}") for i in range(2)]
            # Wab16_i = [W_i | W_i@a2] in fp16 (single mm1 rhs; one
            # accumulation group per PSUM bank -- start=True clears the bank)
            wab16 = [const.tile([128, FOUT + 1], F16, name=f"wab16_{i}", tag=f"wab16_{i}") for i in range(2)]
            for i in range(2):
                t1 = wsm.tile([128, FOUT], DT, name="wtmp", tag="wtmp")
                nc.vector.tensor_tensor(t1[:], Wsb[i][:], abc[:, 0:FOUT], op=ALU.mult)
                nc.vector.reduce_sum(wa1[i][:, 0:1], t1[:], axis=mybir.AxisListType.X)
                t2 = wsm.tile([128, FOUT], DT, name="wtmp", tag="wtmp")
                nc.vector.tensor_tensor(t2[:], Wsb[i][:], abc[:, FOUT:2 * FOUT], op=ALU.mult)
                nc.vector.tensor_copy(wab16[i][:, 0:FOUT], Wsb[i][:])
                wa2s = wsm.tile([128, 1], DT, name="wa2s", tag="wa2s")
                nc.vector.reduce_sum(wa2s[:, 0:1], t2[:], axis=mybir.AxisListType.X)
                nc.vector.tensor_copy(wab16[i][:, FOUT:FOUT + 1], wa2s[:])

            # ---- Wh1 row = a1^T W^T hT (f32 PE) -> fp16 broadcast ----
            w1ps = pbig.tile([1, N], DT, name="big", tag="big")
            for ch in range(4):
                for i in range(2):
                    nc.tensor.matmul(
                        w1ps[0:1, ch * 512:(ch + 1) * 512],
                        wa1[i][:],
                        hT[i][:, ch * 512:(ch + 1) * 512],
                        start=(i == 0),
                        stop=(i == 1),
                    )
            w1row16 = const.tile([1, N], F16, name="w1row16", tag="w1row16")
            nc.vector.tensor_copy(w1row16[:], w1ps[:])
            wh1b16 = const.tile([128, N], F16, name="wh1b16", tag="wh1b16")
            nc.gpsimd.partition_broadcast(wh1b16[:], w1row16[0:1, :])

            den = const.tile([128, NT], DT, name="den", tag="den")
            outp = pbig.tile([FOUT, N], DT, name="big", tag="big")

            D_DVE = N - c_act

            def emit_body():
                # Software-pipelined over m-tiles: "front" stages (mm1, wh2,
                # mask DMA, leaky) run for tile mt while "back" stages
                # (mask-add, exp, 1/den fold, mm2) run for tile mt-1.  This
                # keeps ACT busy (Prelu of mt overlaps the DVE mask-add that
                # gates exp of mt-1) instead of stalling on the per-tile
                # ACT->DVE->ACT round trip.
                stash = {}
                for mt in range(NT + lag):
                    if mt < NT:
                        ms = mt * 128
                        # mm1: Wh tile [128m, 65] = [Wh | Wh@a2] (fp16 inputs)
                        whps = psmall.tile([128, FOUT + 1], DT, name="whps", tag="whps")
                        for i in range(2):
                            nc.tensor.matmul(
                                whps[:, 0:FOUT + 1],
                                hT16[i][:, ms:ms + 128],
                                wab16[i][:],
                                start=(i == 0),
                                stop=(i == 1),
                            )
                        wh2 = wsm.tile([128, 1], DT, name="wh2", tag="wh2")
                        if pool_small:
                            nc.gpsimd.tensor_copy(wh2[:], whps[:, FOUT:FOUT + 1])
                        else:
                            nc.vector.tensor_copy(wh2[:], whps[:, FOUT:FOUT + 1])

                        # mask tile load (plain HWDGE, fp16 {0, -57344})
                        mk = mpool.tile([128, N], F16, name="mk", tag="mk")
                        if diag != "nodma":
                            dma_eng = nc.scalar if (dual_q and mt % 2) else nc.sync
                            dma_eng.dma_start(mk[:], maskT_d.ap()[ms:ms + 128, :])

                        # l = leaky(Wh1[n] + Wh2[m]):
                        #   ACT band: fused Prelu(wh1b16 + wh2)
                        #   DVE band: t = wh1b+wh2 (4x), v = 0.2t (4x), max (2x)
                        l = lpool.tile([128, N], F16, name="l", tag="l")
                        if c_act and diag != "noact":
                            nc.scalar.activation(
                                l[:, 0:c_act], wh1b16[:, 0:c_act], AF.Prelu,
                                bias=wh2[:, 0:1], scale=1.0, alpha=ALPHA,
                            )
                        if diag != "nodve" and D_DVE:
                            t16 = tpool.tile([128, D_DVE], F16, name="t16", tag="t16")
                            nc.vector.tensor_scalar_add(t16[:], wh1b16[:, c_act:], wh2[:, 0:1])
                            v16 = tpool.tile([128, D_DVE], F16, name="v16", tag="v16")
                            nc.vector.tensor_scalar_mul(v16[:], t16[:], ALPHA)
                            nc.vector.tensor_tensor(l[:, c_act:], t16[:], v16[:], op=ALU.max)
                        stash[mt] = (whps, mk, l)

                    if mt >= lag:
                        pt_ = mt - lag
                        whps, mk, l = stash.pop(pt_)
                        # att = l + mask (2x tensor_tensor; optional gpsimd band)
                        att = apool.tile([128, N], F16, name="att", tag="att")
                        if diag == "nodve":
                            att = l
                        elif pool_y:
                            nc.gpsimd.tensor_tensor(att[:, 0:pool_y], l[:, 0:pool_y], mk[:, 0:pool_y], op=ALU.add)
                            nc.vector.tensor_tensor(att[:, pool_y:], l[:, pool_y:], mk[:, pool_y:], op=ALU.add)
                        else:
                            nc.vector.tensor_tensor(att[:], l[:], mk[:], op=ALU.add)

                        # P = exp(att) -> bf16, den = row-sum (f32)
                        ptile = ppool.tile([128, N], BF16, name="pt", tag="pt")
                        nc.scalar.activation(
                            ptile[:], att[:],
                            AF.Identity if diag == "noact" else AF.Exp,
                            accum_out=den[:, pt_:pt_ + 1],
                        )

                        # fold 1/den into Wh -> bf16
                        rc = wsm.tile([128, 1], DT, name="rc", tag="rc")
                        nc.vector.reciprocal(rc[:], den[:, pt_:pt_ + 1])
                        whp = wsm.tile([128, FOUT], BF16, name="whp", tag="whp")
                        if pool_small:
                            nc.gpsimd.tensor_scalar_mul(whp[:], whps[:, 0:FOUT], rc[:, 0:1])
                        else:
                            nc.vector.tensor_scalar_mul(whp[:], whps[:, 0:FOUT], rc[:, 0:1])

                        # mm2: out_T[o, n] += whp.T @ P
                        for ch in range(4 if diag != "nomm2" else 0):
                            nc.tensor.matmul(
                                outp[:, ch * 512:(ch + 1) * 512],
                                whp[:],
                                ptile[:, ch * 512:(ch + 1) * 512],
                                start=(pt_ == 0),
                                stop=(pt_ == NT - 1),
                            )

                # ---- ELU tail: elu(x) = max(x, min(exp(x)-1, 0)) ----
                # fp16 throughout: exp(x) may overflow to +inf for x >> 0,
                # but min(inf - 1, 0) == 0 keeps the identity exact there.
                t_ = wout.tile([FOUT, N], F16, name="t", tag="t")
                q_ = wout.tile([FOUT, N], F16, name="q", tag="q")
                osb = wout.tile([FOUT, N], F16, name="osb", tag="osb")
                nc.scalar.activation(t_[:], outp[:], AF.Exp)
                nc.vector.tensor_scalar(
                    q_[:], t_[:], -1.0, 0.0, op0=ALU.add, op1=ALU.min,
                )
                nc.vector.tensor_tensor(osb[:], outp[:], q_[:], op=ALU.max)
                nc.sync.dma_start(out_d.ap(), osb[:])

            if loop_k:
                with tc.For_i(0, loop_k, 1):
                    for _ in range(reps):
                        emit_body()
            else:
                for _ in range(reps):
                    emit_body()

    nc.compile()
    return nc


def prepare_in_maps(h, adj, W, a):
    h = np.asarray(h, dtype=np.float32)
    adj = np.asarray(adj)
    W = np.asarray(W, dtype=np.float32)
    a = np.asarray(a, dtype=np.float32)
    # maskT[b, m, n] = 0 where adj[b, n, m] > 0 else -57344  (fp16)
    lut = np.array([-MASK, 0.0], dtype=np.float16)
    maskT = lut[adj.transpose(0, 2, 1)]
    in_maps = []
    for b in range(B):
        in_maps.append(
            {
                "hT": np.ascontiguousarray(h[b].T),
                "W": np.ascontiguousarray(W[b]),
                "arow": np.ascontiguousarray(a[b].reshape(1, 2 * FOUT)),
                "maskT": np.ascontiguousarray(maskT[b]),
            }
        )
    return in_maps


def kernel(h, adj, W, a):
    """Full-input entry point: returns elu-GAT output [8, 2048, 64] float32."""
    if "nc" not in _CACHE:
        _CACHE["nc"] = build_program()
    nc = _CACHE["nc"]
    in_maps = prepare_in_maps(h, adj, W, a)
    res = bass_utils.run_bass_kernel_spmd(nc, in_maps, core_ids=list(range(B)))
    out = np.stack([res.results[b]["out"].T for b in range(B)])
    return np.ascontiguousarray(out.astype(np.float32))


# revision 5
# speedup vs baseline: 1.0072x; 1.0072x over previous
"""GraphAttentionLayer (GAT) Bass kernel for Trainium2, 8 NeuronCores — v2.

Problem: B=8, N=2048, Fin=256, Fout=64
    Wh  = h @ W                                   [B, N, 64]
    e   = Wh@a1 + (Wh@a2)^T  (additive scores)    [B, N, N]
    att = where(adj>0, leaky_relu(e, 0.2), -9e15)
    A   = softmax(att, axis=1)   (column softmax!)
    out = elu(A @ Wh)

Sharding: batch-parallel, one graph per core (no communication).

Design vs the accum-DMA baseline (measured ~6-15 ms/iter on HW; this
kernel measures ~78 us/iter):
  * adjacency mask is preloaded host-side as fp16 {0, -57344} and applied
    additively AFTER the leaky-relu (leaky(e) - 57344 <= -57000 so exp()
    still yields exactly 0 for masked entries).  The mask lands via plain
    full-bandwidth HWDGE loads (alternating between the SP and ACT DMA
    queues) + one DVE tensor_tensor add — the slow GPSIMD accumulate-DMA
    is gone entirely.
  * leaky-relu is band-split: columns [0:C_ACT) run as a single fused ACT
    Prelu (bias = per-partition Wh2), the rest on DVE as two 4x-mode
    tensor_scalar ops + one 2x-mode tensor_tensor max (fp16 operands).
  * the m-tile loop is software-pipelined (front stages of tile mt overlap
    back stages of tile mt-1) so ACT never stalls on the per-tile
    ACT->DVE->ACT round trip.
  * exp output P and the folded Wh operand are bf16 (wide exponent: no
    overflow for exp(+20), no subnormal flush at the softmax tail), making
    mm2 a full-rate 1-cycle/row PE matmul. mm1 runs in fp16.
  * ELU tail uses elu(x) = max(x, min(exp(x)-1, 0)) — one ACT pass + two
    fp16 DVE passes; output is stored fp16 and widened on the host.

Per-core dataflow (transposed layout, m = attended-over node on partitions,
n = output node along free axis; m-tiles of 128):
    mm1 (PE):   whps[m, 0:64] = hT16.T @ W16 ; col 64 = Wh@a2   (PSUM f32)
    DMA (SP):   mk[m, n] <- maskT fp16 {0, -57344}
    leaky:      l[:, 0:C]  = ACT Prelu(wh1b16 + wh2)
                l[:, C:]   = DVE max(t, 0.2t), t = wh1b16 + wh2
    mask (DVE): att = l + mk                     (tensor_tensor, 2x mode)
    ACT:        P = Exp(att) -> bf16, accum_out -> den[m] (f32)
    DVE:        rc = 1/den ; whp = Wh[m, 0:64] * rc -> bf16
    mm2 (PE):   out_T[o, n] += whp.T @ P   (accumulate over 16 m-tiles)
    elu tail:   out = max(x, min(exp(x)-1, 0))   (fp16 out, widened on host)
Host: transposes h/adj per batch, encodes adj as fp16 mask, transposes the
output back.

Exactness: softmax without max-subtraction is algebraically identical (exp
stays inside f32/bf16 range), masked entries produce exp(l - 57344) == 0
exactly, and the 1/den fold is applied to the contraction operand Wh.
"""

import sys

import numpy as np

if "/opt/trn_rl_repo" not in sys.path:
    sys.path.append("/opt/trn_rl_repo")

import concourse.bass as bass
import concourse.bacc as bacc
import concourse.mybir as mybir
import concourse.tile as tile
from concourse import bass_utils

B = 8
N = 2048
FIN = 256
FOUT = 64
NT = N // 128          # 16 m-tiles
ALPHA = 0.2
MASK = 57344.0         # exp(x - 57344) == 0 for x in leaky-score range
C_ACT = 1152           # leaky-relu columns fused into ACT Prelu; rest on DVE
POOL_Y = 0             # mask-add columns handled by gpsimd (0 = all on DVE)
DT = mybir.dt.float32
F16 = mybir.dt.float16
BF16 = mybir.dt.bfloat16
AF = mybir.ActivationFunctionType
ALU = mybir.AluOpType

_CACHE = {}


def build_program(reps: int = 1, loop_k: int = 0, c_act: int = C_ACT, pool_y: int = POOL_Y,
                  pool_small: bool = False, diag: str = "", lag: int = 1,
                  dual_q: bool = True):
    """Build and compile the SPMD single-core program (identical on 8 cores).

    reps statically unrolls the main body; loop_k wraps it in a dynamic
    For_i loop instead (constant program size -- used for timing).
    """
    nc = bacc.Bacc(
        "TRN2",
        target_bir_lowering=False,
        debug=False,
        enable_asserts=False,
        num_devices=B,
    )
    hT_d = nc.dram_tensor("hT", [FIN, N], DT, kind="ExternalInput")
    W_d = nc.dram_tensor("W", [FIN, FOUT], DT, kind="ExternalInput")
    arow_d = nc.dram_tensor("arow", [1, 2 * FOUT], DT, kind="ExternalInput")
    maskT_d = nc.dram_tensor("maskT", [N, N], F16, kind="ExternalInput")
    out_d = nc.dram_tensor("out", [FOUT, N], F16, kind="ExternalOutput")

    with tile.TileContext(nc) as tc:
        with (
            tc.tile_pool(name="const", bufs=1) as const,
            tc.tile_pool(name="psmall", bufs=3, space=bass.MemorySpace.PSUM) as psmall,
            tc.tile_pool(name="pbig", bufs=1, space=bass.MemorySpace.PSUM) as pbig,
            tc.tile_pool(name="mpool", bufs=3) as mpool,
            tc.tile_pool(name="tpool", bufs=2) as tpool,
            tc.tile_pool(name="lpool", bufs=2) as lpool,
            tc.tile_pool(name="apool", bufs=2) as apool,
            tc.tile_pool(name="ppool", bufs=3) as ppool,
            tc.tile_pool(name="wsm", bufs=4) as wsm,
            tc.tile_pool(name="wout", bufs=1) as wout,
        ):
            # ---- load inputs ----
            hT = [const.tile([128, N], DT, name=f"hT{i}", tag=f"hT{i}") for i in range(2)]
            Wsb = [const.tile([128, FOUT], DT, name=f"W{i}", tag=f"W{i}") for i in range(2)]
            arow = const.tile([1, 2 * FOUT], DT, name="arow", tag="arow")
            for i in range(2):
                nc.sync.dma_start(hT[i][:], hT_d.ap()[i * 128:(i + 1) * 128, :])
                nc.sync.dma_start(Wsb[i][:], W_d.ap()[i * 128:(i + 1) * 128, :])
            nc.sync.dma_start(arow[:], arow_d.ap())

            # fp16 copies for the mm1 operands (1-cycle/row PE path)
            hT16 = [const.tile([128, N], F16, name=f"hT16_{i}", tag=f"hT16_{i}") for i in range(2)]
            for i in range(2):
                nc.vector.tensor_copy(hT16[i][:], hT[i][:])

            # ---- a broadcast + wa vectors ----
            abc = const.tile([128, 2 * FOUT], DT, name="abc", tag="abc")
            nc.gpsimd.partition_broadcast(abc[:], arow[0:1, :])
            wa1 = [const.tile([128, 1], DT, name=f"wa1_{i}", tag=f"wa1_{i}") for i in range(2)]
            # Wab16_i = [W_i | W_i@a2] in fp16 (single mm1 rhs; one
            # accumulation group per PSUM bank -- start=True clears the bank)
            wab16 = [const.tile([128, FOUT + 1], F16, name=f"wab16_{i}", tag=f"wab16_{i}") for i in range(2)]
            for i in range(2):
                t1 = wsm.tile([128, FOUT], DT, name="wtmp", tag="wtmp")
                nc.vector.tensor_tensor(t1[:], Wsb[i][:], abc[:, 0:FOUT], op=ALU.mult)
                nc.vector.reduce_sum(wa1[i][:, 0:1], t1[:], axis=mybir.AxisListType.X)
                t2 = wsm.tile([128, FOUT], DT, name="wtmp", tag="wtmp")
                nc.vector.tensor_tensor(t2[:], Wsb[i][:], abc[:, FOUT:2 * FOUT], op=ALU.mult)
                nc.vector.tensor_copy(wab16[i][:, 0:FOUT], Wsb[i][:])
                wa2s = wsm.tile([128, 1], DT, name="wa2s", tag="wa2s")
                nc.vector.reduce_sum(wa2s[:, 0:1], t2[:], axis=mybir.AxisListType.X)
                nc.vector.tensor_copy(wab16[i][:, FOUT:FOUT + 1], wa2s[:])

            # ---- Wh1 row = a1^T W^T hT (f32 PE) -> fp16 broadcast ----
            w1ps = pbig.tile([1, N], DT, name="big", tag="big")
            for ch in range(4):
                for i in range(2):
                    nc.tensor.matmul(
                        w1ps[0:1, ch * 512:(ch + 1) * 512],
                        wa1[i][:],
                        hT[i][:, ch * 512:(ch + 1) * 512],
                        start=(i == 0),
                        stop=(i == 1),
                    )
            w1row16 = const.tile([1, N], F16, name="w1row16", tag="w1row16")
            nc.vector.tensor_copy(w1row16[:], w1ps[:])
            wh1b16 = const.tile([128, N], F16, name="wh1b16", tag="wh1b16")
            nc.gpsimd.partition_broadcast(wh1b16[:], w1row16[0:1, :])

            den = const.tile([128, NT], DT, name="den", tag="den")
            outp = pbig.tile([FOUT, N], DT, name="big", tag="big")

            D_DVE = N - c_act

            def emit_body():
                # Software-pipelined over m-tiles: "front" stages (mm1, wh2,
                # mask DMA, leaky) run for tile mt while "back" stages
                # (mask-add, exp, 1/den fold, mm2) run for tile mt-1.  This
                # keeps ACT busy (Prelu of mt overlaps the DVE mask-add that
                # gates exp of mt-1) instead of stalling on the per-tile
                # ACT->DVE->ACT round trip.
                stash = {}
                for mt in range(NT + lag):
                    if mt < NT:
                        ms = mt * 128
                        # mm1: Wh tile [128m, 65] = [Wh | Wh@a2] (fp16 inputs)
                        whps = psmall.tile([128, FOUT + 1], DT, name="whps", tag="whps")
                        for i in range(2):
                            nc.tensor.matmul(
                                whps[:, 0:FOUT + 1],
                                hT16[i][:, ms:ms + 128],
                                wab16[i][:],
                                start=(i == 0),
                                stop=(i == 1),
                            )
                        wh2 = wsm.tile([128, 1], DT, name="wh2", tag="wh2")
                        if pool_small:
                            nc.gpsimd.tensor_copy(wh2[:], whps[:, FOUT:FOUT + 1])
                        else:
                            nc.vector.tensor_copy(wh2[:], whps[:, FOUT:FOUT + 1])

                        # mask tile load (plain HWDGE, fp16 {0, -57344})
                        mk = mpool.tile([128, N], F16, name="mk", tag="mk")
                        if diag != "nodma":
                            dma_eng = nc.scalar if (dual_q and mt % 2) else nc.sync
                            dma_eng.dma_start(mk[:], maskT_d.ap()[ms:ms + 128, :])

                        # l = leaky(Wh1[n] + Wh2[m]):
                        #   ACT band: fused Prelu(wh1b16 + wh2)
                        #   DVE band: t = wh1b+wh2 (4x), v = 0.2t (4x), max (2x)
                        l = lpool.tile([128, N], F16, name="l", tag="l")
                        if c_act and diag != "noact":
                            nc.scalar.activation(
                                l[:, 0:c_act], wh1b16[:, 0:c_act], AF.Prelu,
                                bias=wh2[:, 0:1], scale=1.0, alpha=ALPHA,
                            )
                        if diag != "nodve" and D_DVE:
                            t16 = tpool.tile([128, D_DVE], F16, name="t16", tag="t16")
                            nc.vector.tensor_scalar_add(t16[:], wh1b16[:, c_act:], wh2[:, 0:1])
                            v16 = tpool.tile([128, D_DVE], F16, name="v16", tag="v16")
                            nc.vector.tensor_scalar_mul(v16[:], t16[:], ALPHA)
                            nc.vector.tensor_tensor(l[:, c_act:], t16[:], v16[:], op=ALU.max)
                        stash[mt] = (whps, mk, l)

                    if mt >= lag:
                        pt_ = mt - lag
                        whps, mk, l = stash.pop(pt_)
                        # att = l + mask (2x tensor_tensor; optional gpsimd band)
                        att = apool.tile([128, N], F16, name="att", tag="att")
                        if diag == "nodve":
                            att = l
                        elif pool_y:
                            nc.gpsimd.tensor_tensor(att[:, 0:pool_y], l[:, 0:pool_y], mk[:, 0:pool_y], op=ALU.add)
                            nc.vector.tensor_tensor(att[:, pool_y:], l[:, pool_y:], mk[:, pool_y:], op=ALU.add)
                        else:
                            nc.vector.tensor_tensor(att[:], l[:], mk[:], op=ALU.add)

                        # P = exp(att) -> bf16, den = row-sum (f32)
                        ptile = ppool.tile([128, N], BF16, name="pt", tag="pt")
                        nc.scalar.activation(
                            ptile[:], att[:],
                            AF.Identity if diag == "noact" else AF.Exp,
                            accum_out=den[:, pt_:pt_ + 1],
                        )

                        # fold 1/den into Wh -> bf16
                        rc = wsm.tile([128, 1], DT, name="rc", tag="rc")
                        nc.vector.reciprocal(rc[:], den[:, pt_:pt_ + 1])
                        whp = wsm.tile([128, FOUT], BF16, name="whp", tag="whp")
                        if pool_small:
                            nc.gpsimd.tensor_scalar_mul(whp[:], whps[:, 0:FOUT], rc[:, 0:1])
                        else:
                            nc.vector.tensor_scalar_mul(whp[:], whps[:, 0:FOUT], rc[:, 0:1])

                        # mm2: out_T[o, n] += whp.T @ P
                        for ch in range(4 if diag != "nomm2" else 0):
                            nc.tensor.matmul(
                                outp[:, ch * 512:(ch + 1) * 512],
                                whp[:],
                                ptile[:, ch * 512:(ch + 1) * 512],
                                start=(pt_ == 0),
                                stop=(pt_ == NT - 1),
                            )

                # ---- ELU tail: elu(x) = max(x, min(exp(x)-1, 0)) ----
                # fp16 throughout: exp(x) may overflow to +inf for x >> 0,
                # but min(inf - 1, 0) == 0 keeps the identity exact there.
                t_ = wout.tile([FOUT, N], F16, name="t", tag="t")
                q_ = wout.tile([FOUT, N], F16, name="q", tag="q")
                osb = wout.tile([FOUT, N], F16, name="osb", tag="osb")
                nc.scalar.activation(t_[:], outp[:], AF.Exp)
                nc.vector.tensor_scalar(
                    q_[:], t_[:], -1.0, 0.0, op0=ALU.add, op1=ALU.min,
                )
                nc.vector.tensor_tensor(osb[:], outp[:], q_[:], op=ALU.max)
                nc.sync.dma_start(out_d.ap(), osb[:])

            if loop_k:
                with tc.For_i(0, loop_k, 1):
                    for _ in range(reps):
                        emit_body()
            else:
                for _ in range(reps):
                    emit_body()

    nc.compile()
    return nc


def prepare_in_maps(h, adj, W, a):
    h = np.asarray(h, dtype=np.float32)
    adj = np.asarray(adj)
    W = np.asarray(W, dtype=np.float32)
    a = np.asarray(a, dtype=np.float32)
    # maskT[b, m, n] = 0 where adj[b, n, m] > 0 else -57344  (fp16)
    lut = np.array([-MASK, 0.0], dtype=np.float16)
    maskT = lut[adj.transpose(0, 2, 1)]
    in_maps = []
    for b in range(B):
        in_maps.append(
            {
                "hT": np.ascontiguousarray(h[b].T),
                "W": np.ascontiguousarray(W[b]),
                "arow": np.ascontiguousarray(a[b].reshape(1, 2 * FOUT)),
                "maskT": np.ascontiguousarray(maskT[b]),
            }
        )
    return in_maps


def kernel(h, adj, W, a):
    """Full-input entry point: returns elu-GAT output [8, 2048, 64] float32."""
    if "nc" not in _CACHE:
        _CACHE["nc"] = build_program()
    nc = _CACHE["nc"]
    in_maps = prepare_in_maps(h, adj, W, a)
    res = bass_utils.run_bass_kernel_spmd(nc, in_maps, core_ids=list(range(B)))
    out = np.stack([res.results[b]["out"].T for b in range(B)])
    return np.ascontiguousarray(out.astype(np.float32))
